# revision 1
# baseline (speedup 1.0000x reference)
"""AttentionRNN Trainium2 kernel: 8-core SPMD, vocab-split fc projection.

Self-contained: kernel(**inputs) takes full inputs, returns full [B,S,V] output.
Strategy: every core runs the identical embed+xproj+RNN+attention program
(replicated; the RNN scan is latency-bound so data-parallelism would not help),
and computes a 1/8 vocab slice of the final fc projection (the dominant cost,
537 GFLOP total). No collectives needed; host concatenates the vocab slices.
All matmuls in bf16 with f32 PSUM accumulation (measured end-to-end rel err
~3.5e-3 vs f32 reference).
"""
import sys
if '/opt/trn_rl_repo' not in sys.path:
    sys.path.insert(0, '/opt/trn_rl_repo')

import numpy as np
import ml_dtypes

import concourse.bass as bass
import concourse.mybir as mybir
import concourse.tile as tile
from concourse import bacc
from concourse.bass_utils import run_bass_kernel_spmd
from concourse.masks import make_identity

DT = mybir.dt
BF = DT.bfloat16
F32 = DT.float32
BF_NP = ml_dtypes.bfloat16

VOCAB, H, B, S = 32000, 512, 16, 512
NCORES = 8
VS = VOCAB // NCORES          # 4000 vocab rows per core
TOK = B * S                   # 8192 tokens, order tok = t*16 + b
KH = H // 128                 # 4 h-chunks
KD = (2 * H) // 128           # 8 d-chunks of combined
FC_VW = 512                   # fc vocab chunk width
NVB = (VS + FC_VW - 1) // FC_VW  # fc vocab chunks per core

# debug dump selector: subset of {"uT", "hsT", "ctxT"}
DEBUG_DUMPS = ()
PHASES = 4


def _vb_width(vb):
    return min(512, VS - vb * 512)


def build_nc(phases=PHASES, dumps=DEBUG_DUMPS, repeat=1):
    nc = bacc.Bacc("TRN2", target_bir_lowering=False, debug=False,
                   num_devices=NCORES)

    emb_bf = nc.dram_tensor("emb_bf", [VOCAB, H], BF, kind="ExternalInput").ap()
    idxw = nc.dram_tensor("idxw", [128, TOK // 16], DT.int16, kind="ExternalInput").ap()
    wxhT = nc.dram_tensor("wxhT", [128, KH * H], BF, kind="ExternalInput").ap()
    whhT = nc.dram_tensor("whhT", [128, KH * H], BF, kind="ExternalInput").ap()
    biasT = nc.dram_tensor("biasT", [128, KH], F32, kind="ExternalInput").ap()
    maskT = nc.dram_tensor("maskT", [128, 128], F32, kind="ExternalInput").ap()
    fcwT = nc.dram_tensor("fcwT", [128, NVB * KD * FC_VW], BF, kind="ExternalInput").ap()
    fcb = nc.dram_tensor("fcb", [128, VS], F32, kind="ExternalInput").ap()
    if phases >= 4:
        y = nc.dram_tensor("y", [B, S, VS], F32, kind="ExternalOutput").ap()
    dump_aps = {}
    for name in dumps:
        dump_aps[name] = nc.dram_tensor(
            name + "_dump", [128, KH * TOK], BF, kind="ExternalOutput").ap()

    NT = 512                  # tok chunk for gather + xproj
    NCH = TOK // NT           # 16 chunks
    NSC = 32                  # RNN steps per streamed u chunk
    NUC = S // NSC            # u chunks
    VW = FC_VW                # fc vocab chunk width
    NVB2 = NVB

    with tile.TileContext(nc) as tc:
      for _rep in range(repeat):
        # u = xproj + biases round-trips through HBM so the RNN phase leaves
        # enough SBUF for the attention/fc pools to coexist (streaming).
        u_dram = nc.dram_tensor(f"u_dram{_rep}", [128, KH * TOK], BF).ap()
        u_dram3 = u_dram.rearrange("p (k n) -> p k n", k=KH)
        with tc.tile_pool(name="perm", bufs=1) as perm:
            hsT = perm.tile([128, KH * TOK], BF, tag="hsT")
            ident = perm.tile([128, 128], BF, tag="ident")
            make_identity(nc, ident[:])

            # [128, KH, TOK] views; free index = t*16+b
            hsT3 = hsT[:].rearrange("p (k n) -> p k n", k=KH)
            hsT4 = hsT[:].rearrange("p (k t b) -> p k t b", k=KH, b=B)
            hsT_t = hsT[:].rearrange("p (k t b) -> p t k b", k=KH, b=B)

            # ---------------- phase 1: gather + xproj (u -> HBM) ----------
            with tc.tile_pool(name="p_x", bufs=1) as p_x, \
                 tc.tile_pool(name="p_u", bufs=3) as p_u:
                xeT = p_x.tile([128, KH * TOK], BF, tag="xeT")
                wxh_sb = p_x.tile([128, KH * H], BF, tag="wxh")
                bias_sb = p_x.tile([128, KH], F32, tag="bias")
                idx_sb = p_x.tile([128, TOK // 16], DT.int16, tag="idx")
                nc.sync.dma_start(out=wxh_sb[:], in_=wxhT[:])
                nc.sync.dma_start(out=bias_sb[:], in_=biasT[:])
                nc.sync.dma_start(out=idx_sb[:], in_=idxw[:])
                # chunk-major gather layout: [p, chunk, k, i] = emb[tok, k*128+p]
                xeT4 = xeT[:].rearrange("p (c k n) -> p c k n", c=NCH, k=KH)

                for c in range(NCH):
                    nc.gpsimd.dma_gather(
                        out_ap=xeT4[:, c],
                        in_ap=emb_bf[:],
                        idxs_ap=idx_sb[:, c * (NT // 16):(c + 1) * (NT // 16)],
                        num_idxs=NT,
                        num_idxs_reg=NT,
                        elem_size=H,
                        transpose=True,
                        single_packet=False,
                    )

                with tc.tile_pool(name="ps_x", bufs=4, space="PSUM") as ps_x:
                    for tci in range(NCH):
                        ustg = p_u.tile([128, KH * NT], BF, tag="ustg")
                        ustg3 = ustg[:].rearrange("p (k n) -> p k n", k=KH)
                        for mg in range(KH):
                            px = ps_x.tile([128, NT], F32, tag="px")
                            for k in range(KH):
                                nc.tensor.matmul(
                                    px[:],
                                    lhsT=wxh_sb[:, k * H + mg * 128:k * H + mg * 128 + 128],
                                    rhs=xeT4[:, tci, k, :],
                                    start=(k == 0), stop=(k == KH - 1),
                                )
                            nc.scalar.activation(
                                ustg3[:, mg], px[:],
                                mybir.ActivationFunctionType.Identity,
                                bias=bias_sb[:, mg:mg + 1],
                            )
                        nc.sync.dma_start(
                            out=u_dram3[:, :, tci * NT:(tci + 1) * NT],
                            in_=ustg3[:, :, :])

            # ---------------- phase 2: RNN scan (u streamed back) ----------
            if phases >= 2:
                with tc.tile_pool(name="p_rnn", bufs=1) as p_rnn, \
                     tc.tile_pool(name="p_ub", bufs=2) as p_ub, \
                     tc.tile_pool(name="ps_r", bufs=1, space="PSUM") as ps_r:
                    whh_sb = p_rnn.tile([128, KH * H], BF, tag="whh")
                    nc.sync.dma_start(out=whh_sb[:], in_=whhT[:])
                    for c in range(NUC):
                        ub = p_ub.tile([128, KH * NSC * B], BF, tag="ub")
                        ub3 = ub[:].rearrange("p (k n) -> p k n", k=KH)
                        nc.scalar.dma_start(
                            out=ub3[:, :, :],
                            in_=u_dram3[:, :, c * NSC * B:(c + 1) * NSC * B])
                        for t in range(c * NSC, (c + 1) * NSC):
                            tl = (t - c * NSC) * B
                            if t == 0:
                                ub_t0 = ub[:].rearrange(
                                    "p (k t b) -> p t k b", k=KH, b=B)
                                nc.scalar.activation(
                                    hsT_t[:, 0], ub_t0[:, 0],
                                    mybir.ActivationFunctionType.Tanh)
                                continue
                            prev = slice((t - 1) * B, t * B)
                            # one psum bank holds all 4 m-chunks [128, 4*16]
                            pm = ps_r.tile([128, KH * B], F32, tag="pr")
                            pm2 = pm[:].rearrange("p (k b) -> p k b", k=KH)
                            for mg in range(KH):
                                nc.tensor.matmul(
                                    pm2[:, mg], lhsT=ident[:],
                                    rhs=ub3[:, mg, tl:tl + B],
                                    start=True, stop=False)
                                for k in range(KH):
                                    nc.tensor.matmul(
                                        pm2[:, mg],
                                        lhsT=whh_sb[:, k * H + mg * 128:k * H + mg * 128 + 128],
                                        rhs=hsT3[:, k, prev],
                                        start=False, stop=(k == KH - 1))
                            nc.scalar.activation(
                                hsT_t[:, t], pm2[:],
                                mybir.ActivationFunctionType.Tanh)

                    if "hsT" in dump_aps:
                        nc.sync.dma_start(out=dump_aps["hsT"][:], in_=hsT[:])

                    # ------- phases 3+4: block-streamed attention + fc -------
                    # tq-blocks of 128 timesteps; block mq only needs hs for
                    # t < (mq+1)*128, so attention + fc for early blocks can
                    # overlap the tail of the RNN (pools coexist with p_rnn).
                    if phases >= 3:
                        TB = 128 * B  # 2048 toks per block
                        with tc.tile_pool(name="ph3", bufs=1) as p3, \
                             tc.tile_pool(name="ctxp", bufs=2) as ctxp, \
                             tc.tile_pool(name="p3w", bufs=2) as p3w, \
                             tc.tile_pool(name="fcw", bufs=2) as pfcw, \
                             tc.tile_pool(name="fco", bufs=2) as pfco, \
                             tc.tile_pool(name="ps_s", bufs=2, space="PSUM") as ps_s, \
                             tc.tile_pool(name="ps_t", bufs=2, space="PSUM") as ps_t, \
                             tc.tile_pool(name="ps_c", bufs=1, space="PSUM") as ps_c, \
                             tc.tile_pool(name="ps_o", bufs=2, space="PSUM") as ps_o:
                            # hs in [tk-part, (chunk, b, h)] layout, per block
                            hs_all = p3.tile([128, KH * B * H], BF, tag="hs_all")
                            hs_all4 = hs_all[:].rearrange(
                                "p (c b h) -> p c b h", c=KH, b=B)
                            mask_sb = p3.tile([128, 128], BF, tag="mask")
                            nc.gpsimd.dma_start(out=mask_sb[:], in_=maskT[:])
                            if phases >= 4:
                                fcb_sb = p3.tile([128, VS], BF, tag="fcb")
                                nc.gpsimd.dma_start(out=fcb_sb[:], in_=fcb[:])
                                fcwT3 = fcwT.rearrange("p (vb x) -> p vb x", vb=NVB)
                                y_r = y.rearrange("b (mt dt) v -> mt dt b v", dt=8)
                            for mq in range(KH):
                                ntk = (mq + 1) * 128
                                ctxb = ctxp.tile([128, KH * TB], BF, tag="ctxb")
                                ctxb3 = ctxb[:].rearrange("p (k n) -> p k n", k=KH)
                                ctxb4 = ctxb[:].rearrange(
                                    "p (k t b) -> p k t b", k=KH, b=B)
                                for b in range(B):
                                    # transpose this block's hs chunk (4 kh)
                                    ptt = ps_t.tile([128, KH * 128], BF, tag="ptt")
                                    ptt3 = ptt[:].rearrange("p (k n) -> p k n", k=KH)
                                    for kh in range(KH):
                                        nc.tensor.transpose(
                                            ptt3[:, kh],
                                            hsT4[:, kh, mq * 128:(mq + 1) * 128, b],
                                            ident[:])
                                    nc.vector.tensor_copy(
                                        hs_all4[:, mq, b, :], ptt[:])
                                    # scores, tk <= ntk only (causal skip)
                                    ps = ps_s.tile([128, S], F32, tag="ps")
                                    for kh in range(KH):
                                        nc.tensor.matmul(
                                            ps[:, 0:ntk],
                                            lhsT=hsT4[:, kh, mq * 128:(mq + 1) * 128, b],
                                            rhs=hsT4[:, kh, 0:ntk, b],
                                            start=(kh == 0), stop=(kh == KH - 1))
                                    # mask diag block in place in psum
                                    nc.vector.tensor_tensor(
                                        out=ps[:, mq * 128:ntk],
                                        in0=ps[:, mq * 128:ntk],
                                        in1=mask_sb[:], op=mybir.AluOpType.add)
                                    st = p3w.tile([128, 4], F32, tag="st")
                                    nmx, zs, zi = st[:, 0:1], st[:, 1:2], st[:, 2:3]
                                    nc.vector.reduce_max(
                                        nmx, ps[:, 0:ntk],
                                        axis=mybir.AxisListType.X, negate=True)
                                    es = p3w.tile([128, S], BF, tag="es")
                                    nc.scalar.activation(
                                        es[:, 0:ntk], ps[:, 0:ntk],
                                        mybir.ActivationFunctionType.Exp,
                                        bias=nmx, accum_out=zs)
                                    nc.vector.reciprocal(zi, zs)
                                    w_sb = p3w.tile([128, S], BF, tag="w_sb")
                                    nc.vector.tensor_scalar_mul(
                                        w_sb[:, 0:ntk], es[:, 0:ntk], zi)
                                    # transpose w chunks -> wT [tk-part, 128 tq]
                                    wT = p3w.tile([128, KH * 128], BF, tag="wT")
                                    wT3 = wT[:].rearrange("p (c n) -> p c n", c=KH)
                                    for ktk in range(mq + 1):
                                        pt = ps_t.tile([128, KH * 128], BF,
                                                       tag="ptt", name="pt")
                                        nc.tensor.transpose(
                                            pt[:, 0:128],
                                            w_sb[:, ktk * 128:(ktk + 1) * 128],
                                            ident[:])
                                        nc.vector.tensor_copy(
                                            wT3[:, ktk, :], pt[:, 0:128])
                                    # contextT block cols for b: [(kh) h, tq]
                                    pc = ps_c.tile([128, KH * 128], F32, tag="pc")
                                    pc3 = pc[:].rearrange("p (k n) -> p k n", k=KH)
                                    for mh in range(KH):
                                        for ktk in range(mq + 1):
                                            nc.tensor.matmul(
                                                pc3[:, mh],
                                                lhsT=hs_all4[:, ktk, b,
                                                             mh * 128:(mh + 1) * 128],
                                                rhs=wT3[:, ktk, :],
                                                start=(ktk == 0), stop=(ktk == mq))
                                    nc.vector.tensor_copy(
                                        ctxb4[:, :, :, b], pc3[:, :, :])
                                # fc for this block's 16 token tiles
                                if phases >= 4:
                                    for vb in range(NVB2):
                                        vw = min(VW, VS - vb * VW)
                                        fw = pfcw.tile([128, KD * VW], BF, tag="fw")
                                        fw3 = fw[:].rearrange(
                                            "p (k v) -> p k v", k=KD)
                                        nc.scalar.dma_start(
                                            out=fw[:], in_=fcwT3[:, vb, :])
                                        for mtl in range(TB // 128):
                                            mt = mq * (TB // 128) + mtl
                                            po = ps_o.tile([128, VW], F32, tag="po")
                                            for k in range(KD):
                                                lhsT = (hsT3[:, k, mt * 128:(mt + 1) * 128]
                                                        if k < KH else
                                                        ctxb3[:, k - KH,
                                                              mtl * 128:(mtl + 1) * 128])
                                                nc.tensor.matmul(
                                                    po[:, 0:vw], lhsT=lhsT,
                                                    rhs=fw3[:, k, 0:vw],
                                                    start=(k == 0), stop=(k == KD - 1))
                                            ob = pfco.tile([128, VW], F32, tag="ob")
                                            nc.vector.tensor_tensor(
                                                out=ob[:, 0:vw], in0=po[:, 0:vw],
                                                in1=fcb_sb[:, vb * VW:vb * VW + vw],
                                                op=mybir.AluOpType.add)
                                            nc.sync.dma_start(
                                                out=y_r[mt, :, :, vb * VW:vb * VW + vw],
                                                in_=ob[:, 0:vw])
    nc.compile()
    return nc


# ---------------------------------------------------------------------------
# host side
# ---------------------------------------------------------------------------

def prep_inputs(x, emb, Wxh_w, Wxh_b, Whh_w, Whh_b, fc_w, fc_b):
    """Build per-core in_maps with device layouts."""
    x = np.asarray(x)
    emb = np.asarray(emb, dtype=np.float32)
    Wxh_w = np.asarray(Wxh_w, dtype=np.float32)
    Wxh_b = np.asarray(Wxh_b, dtype=np.float32)
    Whh_w = np.asarray(Whh_w, dtype=np.float32)
    Whh_b = np.asarray(Whh_b, dtype=np.float32)
    fc_w = np.asarray(fc_w, dtype=np.float32)
    fc_b = np.asarray(fc_b, dtype=np.float32)

    emb_bf = np.ascontiguousarray(emb.astype(BF_NP))
    # idx wrapped: flat tok order = t*16+b ; slot j -> [j%16, j//16]
    idx_flat = np.ascontiguousarray(x.T).reshape(-1).astype(np.int64)  # [S*B] t-major
    wrapped = idx_flat.reshape(TOK // 16, 16).T.astype(np.int16)  # [16, TOK//16]
    # replicated across the 8 gpsimd Q7 cores: each reads its own 16-partition group
    idxw = np.ascontiguousarray(np.tile(wrapped, (8, 1)))

    def pack_T(w):  # w [G, H] -> lhsT layout [128, KH*G] : [p, k*G+g] = w[g, k*128+p]
        wT = np.ascontiguousarray(w.T)            # [H, G]
        kh = wT.shape[0] // 128
        return np.ascontiguousarray(
            wT.reshape(kh, 128, wT.shape[1]).transpose(1, 0, 2).reshape(128, -1)
        ).astype(BF_NP)

    wxhT = pack_T(Wxh_w)                          # [128, KH*H]
    whhT = pack_T(Whh_w)
    bias = (Wxh_b + Whh_b).astype(np.float32)
    biasT = np.ascontiguousarray(bias.reshape(KH, 128).T)  # [128, KH]

    p = np.arange(128)[:, None]
    j = np.arange(128)[None, :]
    maskT = np.where(j <= p, 0.0, -1e30).astype(np.float32)

    base = {
        "emb_bf": emb_bf, "idxw": idxw, "wxhT": wxhT, "whhT": whhT,
        "biasT": biasT, "maskT": maskT,
    }
    in_maps = []
    for c in range(NCORES):
        sl = slice(c * VS, (c + 1) * VS)
        fcwT_kv = pack_T(fc_w[sl]).reshape(128, KD, VS)   # [p, k, v]
        # vb-major contiguous: [p, vb, k, FC_VW] (zero-padded last chunk)
        fcwT = np.zeros((128, NVB, KD, FC_VW), BF_NP)
        for vb in range(NVB):
            vw = min(FC_VW, VS - vb * FC_VW)
            fcwT[:, vb, :, :vw] = fcwT_kv[:, :, vb * FC_VW:vb * FC_VW + vw]
        fcwT = np.ascontiguousarray(fcwT.reshape(128, NVB * KD * FC_VW))
        fcb_bc = np.ascontiguousarray(
            np.broadcast_to(fc_b[sl].astype(np.float32), (128, VS)))
        m = dict(base)
        m["fcwT"] = fcwT
        m["fcb"] = fcb_bc
        in_maps.append(m)
    return in_maps


_NC_CACHE = {}


def get_nc(phases=PHASES, dumps=DEBUG_DUMPS):
    key = (phases, tuple(dumps))
    if key not in _NC_CACHE:
        _NC_CACHE[key] = build_nc(phases, dumps)
    return _NC_CACHE[key]


def kernel(x, emb, Wxh_w, Wxh_b, Whh_w, Whh_b, fc_w, fc_b):
    nc = get_nc()
    in_maps = prep_inputs(x, emb, Wxh_w, Wxh_b, Whh_w, Whh_b, fc_w, fc_b)
    res = run_bass_kernel_spmd(nc, in_maps, list(range(NCORES)))
    y = np.concatenate([res.results[c]["y"] for c in range(NCORES)], axis=2)
    return np.ascontiguousarray(y.astype(np.float32))



# revision 19
# speedup vs baseline: 19.7265x; 19.7265x over previous
"""AttentionRNN Trainium2 kernel v2: 8-core SPMD, vocab-split fc projection.

Self-contained: kernel(**inputs) takes full inputs, returns full [B,S,V] output.
Strategy: every core runs the identical embed+xproj+RNN+attention program
(replicated; the RNN scan is latency-bound so batch-parallelism would not
help), and computes a 1/8 vocab slice of the final fc projection (the
dominant cost, 537 GFLOP total). No collectives; host concatenates slices.

v2 over baseline:
  - u (xproj output) stays in SBUF per 128-step block (no HBM round trip)
  - RNN u-injection via DVE add (drops 4 identity matmuls per step)
  - software-pipelined emission: attention+fc of block q-1 is emitted
    between the RNN steps of block q, so the PE stream has independent
    matmuls to chew on while each step waits for its tanh
  - y output in bf16, contiguous [vb, mt, p, v] device layout, stores
    batched 4 token-tiles per DMA, alternating SP / GpSimd queues
"""
import sys
if '/opt/trn_rl_repo' not in sys.path:
    sys.path.insert(0, '/opt/trn_rl_repo')

import numpy as np
import ml_dtypes

import concourse.bass as bass
import concourse.mybir as mybir
import concourse.tile as tile
from concourse import bacc
from concourse.bass_utils import run_bass_kernel_spmd
from concourse.masks import make_identity

DT = mybir.dt
BF = DT.bfloat16
F32 = DT.float32
BF_NP = ml_dtypes.bfloat16

VOCAB, H, B, S = 32000, 512, 16, 512
NCORES = 8
VS = VOCAB // NCORES          # 4000 vocab rows per core
TOK = B * S                   # 8192 tokens, order tok = t*16 + b
KH = H // 128                 # 4 h-chunks
KD = (2 * H) // 128           # 8 d-chunks of combined
VW = 512                      # fc vocab chunk width
NVB = (VS + VW - 1) // VW     # 8 fc vocab chunks per core
NT = 512                      # tokens per gather/xproj chunk
NCH = TOK // NT               # 16 chunks
BLK = 128                     # timesteps per attention block
NBLK = S // BLK               # 4 blocks
TB = BLK * B                  # 2048 tokens per block
CPB = NCH // NBLK             # 4 chunks per block
OB = 4                        # token-tiles batched per y store
MTB = TOK // 128              # 64 token tiles


def build_nc(repeat=1):
    nc = bacc.Bacc("TRN2", target_bir_lowering=False, debug=False,
                   num_devices=NCORES)

    emb_bf = nc.dram_tensor("emb_bf", [VOCAB, H], BF, kind="ExternalInput").ap()
    idxw = nc.dram_tensor("idxw", [128, TOK // 16], DT.int16, kind="ExternalInput").ap()
    wxhT = nc.dram_tensor("wxhT", [128, KH * H], BF, kind="ExternalInput").ap()
    whhT = nc.dram_tensor("whhT", [128, KH * H], BF, kind="ExternalInput").ap()
    biasT = nc.dram_tensor("biasT", [128, KH], F32, kind="ExternalInput").ap()
    maskT = nc.dram_tensor("maskT", [128, 128], F32, kind="ExternalInput").ap()
    fcwT = nc.dram_tensor("fcwT", [128, NVB * KD * VW], BF, kind="ExternalInput").ap()
    fcb_in = nc.dram_tensor("fcb", [128, NVB * VW], BF, kind="ExternalInput").ap()
    y = nc.dram_tensor("y", [NVB, MTB, 128, VW], BF, kind="ExternalOutput").ap()
    y_r = y.rearrange("a m p v -> p a m v")
    fcwT3 = fcwT.rearrange("p (vb x) -> p vb x", vb=NVB)

    with tile.TileContext(nc) as tc:
      for _rep in range(repeat):
        with tc.tile_pool(name="perm", bufs=1) as perm, \
             tc.tile_pool(name="p_xe", bufs=2) as p_xe, \
             tc.tile_pool(name="p_u", bufs=2) as p_u, \
             tc.tile_pool(name="p_hz", bufs=2) as p_hz, \
             tc.tile_pool(name="p_fw", bufs=2) as p_fw, \
             tc.tile_pool(name="p_fcb", bufs=2) as p_fcb, \
             tc.tile_pool(name="p_ctx", bufs=2) as p_ctx, \
             tc.tile_pool(name="p_hb", bufs=2) as p_hb, \
             tc.tile_pool(name="p_att", bufs=2) as p_att, \
             tc.tile_pool(name="p_yt", bufs=2) as p_yt, \
             tc.tile_pool(name="ps_o", bufs=2, space="PSUM") as ps_o, \
             tc.tile_pool(name="ps_m", bufs=2, space="PSUM") as ps_m, \
             tc.tile_pool(name="ps_s", bufs=2, space="PSUM") as ps_s, \
             tc.tile_pool(name="ps_t", bufs=2, space="PSUM") as ps_t:
            hsT = perm.tile([128, KH * TOK], BF, tag="hsT")
            ident = perm.tile([128, 128], BF, tag="ident")
            make_identity(nc, ident[:])
            hsT3 = hsT[:].rearrange("p (k n) -> p k n", k=KH)
            hsT4 = hsT[:].rearrange("p (k t b) -> p k t b", k=KH, b=B)
            hsT_t = hsT[:].rearrange("p (k t b) -> p t k b", k=KH, b=B)

            wxh_sb = perm.tile([128, KH * H], BF, tag="wxh")
            whh_sb = perm.tile([128, KH * H], BF, tag="whh")
            bias_sb = perm.tile([128, KH], F32, tag="bias")
            idx_sb = perm.tile([128, TOK // 16], DT.int16, tag="idx")
            mask_sb = perm.tile([128, 128], BF, tag="mask")
            nc.sync.dma_start(out=wxh_sb[:], in_=wxhT[:])
            nc.sync.dma_start(out=whh_sb[:], in_=whhT[:])
            nc.sync.dma_start(out=bias_sb[:], in_=biasT[:])
            nc.sync.dma_start(out=idx_sb[:], in_=idxw[:])
            nc.gpsimd.dma_start(out=mask_sb[:], in_=maskT[:])

            # per-block u tiles (bufs=2): ub[q] holds xproj for block q
            u_tiles = {}
            xe_tiles = {}

            def u_tile(q):
                if q not in u_tiles:
                    t = p_u.tile([128, KH * TB], BF, tag="ub")
                    u_tiles[q] = (t, t[:].rearrange("p (k n) -> p k n", k=KH))
                return u_tiles[q]

            def emit_gather(c):
                t = p_xe.tile([128, KH * NT], BF, tag="xec")
                xe_tiles[c] = (t, t[:].rearrange("p (k n) -> p k n", k=KH))
                nc.gpsimd.dma_gather(
                    out_ap=xe_tiles[c][0][:].rearrange(
                        "p (k n) -> p k n", k=KH),
                    in_ap=emb_bf[:],
                    idxs_ap=idx_sb[:, c * (NT // 16):(c + 1) * (NT // 16)],
                    num_idxs=NT,
                    num_idxs_reg=NT,
                    elem_size=H,
                    transpose=True,
                    single_packet=False,
                )

            def emit_xproj(c):
                q = c // CPB
                cc = c % CPB
                _, ub3 = u_tile(q)
                _, xe3 = xe_tiles[c]
                for mg in range(KH):
                    px = ps_o.tile([128, VW], F32, tag="po", name="px")
                    for k in range(KH):
                        nc.tensor.matmul(
                            px[:],
                            lhsT=wxh_sb[:, k * H + mg * 128:k * H + mg * 128 + 128],
                            rhs=xe3[:, k, :],
                            start=(k == 0), stop=(k == KH - 1),
                        )
                    nc.scalar.activation(
                        ub3[:, mg, cc * NT:(cc + 1) * NT], px[:],
                        mybir.ActivationFunctionType.Identity,
                        bias=bias_sb[:, mg:mg + 1],
                    )
                del xe_tiles[c]

            def emit_rnn_step(t):
                q = t // BLK
                tl = t % BLK
                _, ub3 = u_tile(q)
                if t == 0:
                    nc.scalar.activation(
                        hsT_t[:, 0], ub3[:, :, 0:B],
                        mybir.ActivationFunctionType.Tanh)
                    return
                prev = slice((t - 1) * B, t * B)
                pm = ps_m.tile([128, KH * B], F32, tag="pm")
                pm2 = pm[:].rearrange("p (k b) -> p k b", k=KH)
                # Block 0 has no fc work to hide engine-hop latency behind, so
                # inject u via an identity matmul (PE-only critical chain).
                # Later blocks use a DVE add instead, saving PE time where PE
                # is the bottleneck.
                use_ident = (q == 0)
                for mg in range(KH):
                    if use_ident:
                        nc.tensor.matmul(
                            pm2[:, mg], lhsT=ident[:],
                            rhs=ub3[:, mg, tl * B:(tl + 1) * B],
                            start=True, stop=False)
                    for k in range(KH):
                        nc.tensor.matmul(
                            pm2[:, mg],
                            lhsT=whh_sb[:, k * H + mg * 128:k * H + mg * 128 + 128],
                            rhs=hsT3[:, k, prev],
                            start=(not use_ident and k == 0),
                            stop=(k == KH - 1))
                if use_ident:
                    nc.scalar.activation(
                        hsT_t[:, t], pm2[:],
                        mybir.ActivationFunctionType.Tanh)
                else:
                    hz = p_hz.tile([128, KH * B], BF, tag="hz")
                    nc.vector.tensor_tensor(
                        out=hz[:], in0=pm[:],
                        in1=ub3[:, :, tl * B:(tl + 1) * B],
                        op=mybir.AluOpType.add)
                    nc.scalar.activation(
                        hsT_t[:, t], hz[:].rearrange("p (k b) -> p k b", k=KH),
                        mybir.ActivationFunctionType.Tanh)

            ctx_tiles = {}

            def copy_eng(b):
                # alternate psum->sbuf copies between DVE and ACT so the
                # attention clump doesn't saturate DVE
                return nc.vector if b % 2 == 0 else nc.scalar

            def eng_copy(eng, out, in_):
                if eng is nc.vector:
                    eng.tensor_copy(out, in_)
                else:
                    eng.activation(out, in_,
                                   mybir.ActivationFunctionType.Identity)

            def emit_attn_b(mq, b):
                ntk = (mq + 1) * 128
                if mq not in ctx_tiles:
                    t = p_ctx.tile([128, KH * TB], BF, tag="ctxb")
                    ctx_tiles[mq] = (
                        t,
                        t[:].rearrange("p (k n) -> p k n", k=KH),
                        t[:].rearrange("p (k t b) -> p k t b", k=KH, b=B),
                    )
                _, _, ctxb4 = ctx_tiles[mq]
                # transpose hs blocks 0..mq for batch b into per-b staging
                # (recomputed per block: cheaper in SBUF than retaining the
                # full transposed hs, which would cost another 64KB/part)
                hb = p_hb.tile([128, KH * 512], BF, tag="hb")
                hb3 = hb[:].rearrange("p (c h) -> p c h", c=KH)
                for ktk in range(mq + 1):
                    ptt = ps_t.tile([128, KH * 128], BF, tag="pmisc", name="ptt")
                    ptt3 = ptt[:].rearrange("p (k n) -> p k n", k=KH)
                    for kh in range(KH):
                        nc.tensor.transpose(
                            ptt3[:, kh],
                            hsT4[:, kh, ktk * 128:(ktk + 1) * 128, b],
                            ident[:])
                    eng_copy(copy_eng(b), hb3[:, ktk], ptt[:])
                # scores (causal skip)
                ps = ps_s.tile([128, S], F32, tag="ps")
                for kh in range(KH):
                    nc.tensor.matmul(
                        ps[:, 0:ntk],
                        lhsT=hsT4[:, kh, mq * 128:(mq + 1) * 128, b],
                        rhs=hsT4[:, kh, 0:ntk, b],
                        start=(kh == 0), stop=(kh == KH - 1))
                nc.vector.tensor_tensor(
                    out=ps[:, mq * 128:ntk], in0=ps[:, mq * 128:ntk],
                    in1=mask_sb[:], op=mybir.AluOpType.add)
                st = p_att.tile([128, 4], F32, tag="st")
                nmx, zs, zi = st[:, 0:1], st[:, 1:2], st[:, 2:3]
                nc.vector.reduce_max(
                    nmx, ps[:, 0:ntk], axis=mybir.AxisListType.X, negate=True)
                w_sb = p_att.tile([128, S], BF, tag="w_sb")
                nc.scalar.activation(
                    w_sb[:, 0:ntk], ps[:, 0:ntk],
                    mybir.ActivationFunctionType.Exp,
                    bias=nmx, accum_out=zs)
                nc.vector.reciprocal(zi, zs)
                nc.vector.tensor_scalar_mul(w_sb[:, 0:ntk], w_sb[:, 0:ntk], zi)
                # transpose w chunks -> wT [tk-part, 128 tq]
                wT = p_att.tile([128, KH * 128], BF, tag="wT")
                wT3 = wT[:].rearrange("p (c n) -> p c n", c=KH)
                for ktk in range(mq + 1):
                    pt = ps_t.tile([128, KH * 128], BF, tag="pmisc", name="pt")
                    nc.tensor.transpose(
                        pt[:, 0:128], w_sb[:, ktk * 128:(ktk + 1) * 128],
                        ident[:])
                    eng_copy(copy_eng(b), wT3[:, ktk, :], pt[:, 0:128])
                # contextT block cols for b
                pc = ps_t.tile([128, KH * 128], F32, tag="pmisc", name="pc")
                pc3 = pc[:].rearrange("p (k n) -> p k n", k=KH)
                for mh in range(KH):
                    for ktk in range(mq + 1):
                        nc.tensor.matmul(
                            pc3[:, mh],
                            lhsT=hb3[:, ktk, mh * 128:(mh + 1) * 128],
                            rhs=wT3[:, ktk, :],
                            start=(ktk == 0), stop=(ktk == mq))
                eng_copy(copy_eng(b), ctxb4[:, :, :, b], pc3[:, :, :])

            fw_tiles = {}
            fcb_tiles = {}

            def emit_fw_load(q, vb):
                fw = p_fw.tile([128, KD * VW], BF, tag="fw")
                fw_tiles[(q, vb)] = (fw, fw[:].rearrange("p (k v) -> p k v", k=KD))
                nc.sync.dma_start(out=fw[:], in_=fcwT3[:, vb, :])
                fcbc = p_fcb.tile([128, VW], BF, tag="fcbc")
                fcb_tiles[(q, vb)] = fcbc
                nc.sync.dma_start(
                    out=fcbc[:], in_=fcb_in[:, vb * VW:(vb + 1) * VW])

            yt_tiles = {}

            def emit_fc_group(q, vb, mtl):
                _, _, _ = 0, 0, 0
                _, fw3 = fw_tiles[(q, vb)]
                fcbc = fcb_tiles[(q, vb)]
                _, ctxb3, _ = ctx_tiles[q]
                mt = q * (TB // 128) + mtl
                po = ps_o.tile([128, VW], F32, tag="po", name="po")
                for k in range(KD):
                    lhsT = (hsT3[:, k, mt * 128:(mt + 1) * 128]
                            if k < KH else
                            ctxb3[:, k - KH, mtl * 128:(mtl + 1) * 128])
                    nc.tensor.matmul(
                        po[:], lhsT=lhsT, rhs=fw3[:, k],
                        start=(k == 0), stop=(k == KD - 1))
                og = mtl // OB
                if (q, vb, og) not in yt_tiles:
                    yt_tiles[(q, vb, og)] = p_yt.tile(
                        [128, OB * VW], BF, tag="yt", name="yt")
                yt = yt_tiles[(q, vb, og)]
                ol = mtl % OB
                nc.vector.tensor_tensor(
                    out=yt[:, ol * VW:(ol + 1) * VW], in0=po[:],
                    in1=fcbc[:], op=mybir.AluOpType.add)

            store_eng = [0]

            def emit_store(q, vb, og):
                yt = yt_tiles.pop((q, vb, og))
                mt0 = q * (TB // 128) + og * OB
                eng = nc.sync if store_eng[0] % 2 == 0 else nc.gpsimd
                store_eng[0] += 1
                eng.dma_start(
                    out=y_r[:, vb, mt0:mt0 + OB, :],
                    in_=yt[:].rearrange("p (m v) -> p m v", m=OB))

            def attn_items(qm):
                return [("attn", lambda mq=qm, b=b: emit_attn_b(mq, b))
                        for b in range(B)]

            def fc_items(qm, vbs, early_fw=2):
                items = []
                for vb in vbs[:early_fw]:
                    items.append(("fw", lambda q=qm, vb=vb: emit_fw_load(q, vb)))
                for i, vb in enumerate(vbs):
                    if i >= early_fw:
                        items.append(
                            ("fw", lambda q=qm, vb=vb: emit_fw_load(q, vb)))
                    for og in range(TB // 128 // OB):
                        for ol in range(OB):
                            items.append(
                                ("fc", lambda q=qm, vb=vb, mtl=og * OB + ol:
                                 emit_fc_group(q, vb, mtl)))
                        items.append(
                            ("st", lambda q=qm, vb=vb, og=og:
                             emit_store(q, vb, og)))
                return items

            def block_work(qm, vbs):
                """Work items for block qm: early fw loads, attention
                (paced 1/step), then fc groups + stores."""
                fitems = fc_items(qm, vbs)
                return fitems[:2] + attn_items(qm) + fitems[2:]

            # ---------------- emission schedule ----------------
            xg = [0, 0]   # next gather, next xproj

            def pump_xproj(n, qmax):
                for _ in range(n):
                    if xg[0] < NCH and xg[0] <= xg[1] + 1 \
                            and xg[0] // CPB <= qmax + 1:
                        emit_gather(xg[0])
                        xg[0] += 1
                    if xg[1] < NCH and xg[1] < xg[0] \
                            and xg[1] // CPB <= qmax:
                        emit_xproj(xg[1])
                        xg[1] += 1

            pump_xproj(2, 0)
            work = []
            for q in range(NBLK):
                if q == 0:
                    work = []
                elif q < NBLK - 1:
                    work = block_work(q - 1, list(range(NVB)))
                else:
                    # hold back the last 2 vb chunks of block 2's fc to
                    # interleave with block 3's attention in the tail
                    work = block_work(q - 1, list(range(NVB - 2)))
                wi = 0
                for cc in range(CPB):
                    pump_xproj(2, q + 1)
                    for tl in range(BLK // CPB):
                        t = (q * CPB + cc) * (BLK // CPB) + tl
                        emit_rnn_step(t)
                        steps_left = BLK - (cc * (BLK // CPB) + tl)
                        want = min(
                            -(-(len(work) - wi) // max(steps_left, 1)), 4)
                        popped = 0
                        while wi < len(work) and popped < want:
                            kind, fn = work[wi]
                            fn()
                            wi += 1
                            popped += 1
                            if kind == "attn":
                                break
                while wi < len(work):
                    work[wi][1]()
                    wi += 1
            # tail: interleave block 3 attention with block 2's held-back fc
            leftover = fc_items(NBLK - 2, [NVB - 2, NVB - 1])
            a3 = attn_items(NBLK - 1)
            li = 0
            for i, (_, it) in enumerate(a3):
                it()
                take = -(-(len(leftover) - li) // (len(a3) - i))
                for _ in range(min(take, 4)):
                    if li < len(leftover):
                        leftover[li][1]()
                        li += 1
            while li < len(leftover):
                leftover[li][1]()
                li += 1
            for _, it in fc_items(NBLK - 1, list(range(NVB))):
                it()

    nc.compile()
    return nc


# ---------------------------------------------------------------------------
# host side
# ---------------------------------------------------------------------------

def prep_inputs(x, emb, Wxh_w, Wxh_b, Whh_w, Whh_b, fc_w, fc_b):
    """Build per-core in_maps with device layouts."""
    x = np.asarray(x)
    emb = np.asarray(emb, dtype=np.float32)
    Wxh_w = np.asarray(Wxh_w, dtype=np.float32)
    Wxh_b = np.asarray(Wxh_b, dtype=np.float32)
    Whh_w = np.asarray(Whh_w, dtype=np.float32)
    Whh_b = np.asarray(Whh_b, dtype=np.float32)
    fc_w = np.asarray(fc_w, dtype=np.float32)
    fc_b = np.asarray(fc_b, dtype=np.float32)

    emb_bf = np.ascontiguousarray(emb.astype(BF_NP))
    # idx wrapped: flat tok order = t*16+b ; slot j -> [j%16, j//16]
    idx_flat = np.ascontiguousarray(x.T).reshape(-1).astype(np.int64)
    wrapped = idx_flat.reshape(TOK // 16, 16).T.astype(np.int16)
    idxw = np.ascontiguousarray(np.tile(wrapped, (8, 1)))

    def pack_T(w):  # w [G, H] -> lhsT layout [128, KH*G]
        wT = np.ascontiguousarray(w.T)            # [H, G]
        kh = wT.shape[0] // 128
        return np.ascontiguousarray(
            wT.reshape(kh, 128, wT.shape[1]).transpose(1, 0, 2).reshape(128, -1)
        ).astype(BF_NP)

    wxhT = pack_T(Wxh_w)
    whhT = pack_T(Whh_w)
    bias = (Wxh_b + Whh_b).astype(np.float32)
    biasT = np.ascontiguousarray(bias.reshape(KH, 128).T)

    p = np.arange(128)[:, None]
    j = np.arange(128)[None, :]
    maskT = np.where(j <= p, 0.0, -1e30).astype(np.float32)

    base = {
        "emb_bf": emb_bf, "idxw": idxw, "wxhT": wxhT, "whhT": whhT,
        "biasT": biasT, "maskT": maskT,
    }
    in_maps = []
    for c in range(NCORES):
        sl = slice(c * VS, (c + 1) * VS)
        fcwT_kv = pack_T(fc_w[sl]).reshape(128, KD, VS)   # [p, k, v]
        fcwT = np.zeros((128, NVB, KD, VW), BF_NP)
        for vb in range(NVB):
            vw = min(VW, VS - vb * VW)
            fcwT[:, vb, :, :vw] = fcwT_kv[:, :, vb * VW:vb * VW + vw]
        fcwT = np.ascontiguousarray(fcwT.reshape(128, NVB * KD * VW))
        fcb_pad = np.zeros(NVB * VW, np.float32)
        fcb_pad[:VS] = fc_b[sl]
        fcb_bc = np.ascontiguousarray(
            np.broadcast_to(fcb_pad.astype(BF_NP), (128, NVB * VW)))
        m = dict(base)
        m["fcwT"] = fcwT
        m["fcb"] = fcb_bc
        in_maps.append(m)
    return in_maps


def unpack_y(res_y_list):
    """res_y_list: per-core y [NVB, MTB, 128, VW] bf16 -> [B, S, V] f32."""
    out = np.empty((B, S, VOCAB), np.float32)
    for c, yd in enumerate(res_y_list):
        yc = np.asarray(yd)                       # [NVB, MTB, 128, VW]
        # tok = mt*128 + p ; tok = t*16 + b
        for vb in range(NVB):
            vw = min(VW, VS - vb * VW)
            v0 = c * VS + vb * VW
            blk = yc[vb].reshape(TOK, VW)[:, :vw].astype(np.float32)
            out[:, :, v0:v0 + vw] = blk.reshape(S, B, vw).transpose(1, 0, 2)
    return out


_NC_CACHE = {}


def get_nc(repeat=1):
    if repeat not in _NC_CACHE:
        _NC_CACHE[repeat] = build_nc(repeat)
    return _NC_CACHE[repeat]


def kernel(x, emb, Wxh_w, Wxh_b, Whh_w, Whh_b, fc_w, fc_b):
    nc = get_nc()
    in_maps = prep_inputs(x, emb, Wxh_w, Wxh_b, Whh_w, Whh_b, fc_w, fc_b)
    res = run_bass_kernel_spmd(nc, in_maps, list(range(NCORES)))
    return unpack_y([res.results[c]["y"] for c in range(NCORES)])


# revision 22
# speedup vs baseline: 55.7047x; 2.8239x over previous
"""AttentionRNN Trainium2 kernel v2: 8-core SPMD, vocab-split fc projection.

Self-contained: kernel(**inputs) takes full inputs, returns full [B,S,V] output.
Strategy: every core runs the identical embed+xproj+RNN+attention program
(replicated; the RNN scan is latency-bound so batch-parallelism would not
help), and computes a 1/8 vocab slice of the final fc projection (the
dominant cost, 537 GFLOP total). No collectives; host concatenates slices.

v2 over baseline:
  - u (xproj output) stays in SBUF per 128-step block (no HBM round trip)
  - RNN u-injection via DVE add (drops 4 identity matmuls per step)
  - software-pipelined emission: attention+fc of block q-1 is emitted
    between the RNN steps of block q, so the PE stream has independent
    matmuls to chew on while each step waits for its tanh
  - y output in bf16, contiguous [vb, mt, p, v] device layout, stores
    batched 4 token-tiles per DMA, alternating SP / GpSimd queues
"""
import sys
if '/opt/trn_rl_repo' not in sys.path:
    sys.path.insert(0, '/opt/trn_rl_repo')

import numpy as np
import ml_dtypes

import concourse.bass as bass
import concourse.mybir as mybir
import concourse.tile as tile
from concourse import bacc
from concourse.bass_utils import run_bass_kernel_spmd
from concourse.masks import make_identity

DT = mybir.dt
BF = DT.bfloat16
F32 = DT.float32
BF_NP = ml_dtypes.bfloat16

VOCAB, H, B, S = 32000, 512, 16, 512
NCORES = 8
VS = VOCAB // NCORES          # 4000 vocab rows per core
TOK = B * S                   # 8192 tokens, order tok = t*16 + b
KH = H // 128                 # 4 h-chunks
KD = (2 * H) // 128           # 8 d-chunks of combined
VW = 512                      # fc vocab chunk width
NVB = (VS + VW - 1) // VW     # 8 fc vocab chunks per core
NT = 512                      # tokens per gather/xproj chunk
NCH = TOK // NT               # 16 chunks
BLK = 128                     # timesteps per attention block
NBLK = S // BLK               # 4 blocks
TB = BLK * B                  # 2048 tokens per block
CPB = NCH // NBLK             # 4 chunks per block
OB = 4                        # token-tiles batched per y store
MTB = TOK // 128              # 64 token tiles


def build_nc(repeat=1):
    nc = bacc.Bacc("TRN2", target_bir_lowering=False, debug=False,
                   num_devices=NCORES)

    emb_bf = nc.dram_tensor("emb_bf", [VOCAB, H], BF, kind="ExternalInput").ap()
    idxw = nc.dram_tensor("idxw", [128, TOK // 16], DT.int16, kind="ExternalInput").ap()
    wxhT = nc.dram_tensor("wxhT", [128, KH * H], BF, kind="ExternalInput").ap()
    whhT = nc.dram_tensor("whhT", [128, KH * H], BF, kind="ExternalInput").ap()
    biasT = nc.dram_tensor("biasT", [128, KH], F32, kind="ExternalInput").ap()
    maskT = nc.dram_tensor("maskT", [128, 128], F32, kind="ExternalInput").ap()
    fcwT = nc.dram_tensor("fcwT", [128, NVB * KD * VW], BF, kind="ExternalInput").ap()
    fcb_in = nc.dram_tensor("fcb", [128, NVB * VW], BF, kind="ExternalInput").ap()
    y = nc.dram_tensor("y", [NVB, MTB, 128, VW], BF, kind="ExternalOutput").ap()
    y_r = y.rearrange("a m p v -> p a m v")
    fcwT3 = fcwT.rearrange("p (vb x) -> p vb x", vb=NVB)

    with tile.TileContext(nc) as tc:
      for _rep in range(repeat):
        # transposed-hs retention: [block, b, t-part, h] in DRAM; blocks < mq
        # are read back instead of re-transposed on PE
        hsat = nc.dram_tensor(f"hsat{_rep}", [NBLK, B, 128, H], BF).ap()
        hsat_r = hsat.rearrange("c b p h -> p c b h")
        with tc.tile_pool(name="perm", bufs=1) as perm, \
             tc.tile_pool(name="p_xe", bufs=2) as p_xe, \
             tc.tile_pool(name="p_u", bufs=2) as p_u, \
             tc.tile_pool(name="p_hz", bufs=2) as p_hz, \
             tc.tile_pool(name="p_fw", bufs=2) as p_fw, \
             tc.tile_pool(name="p_fcb", bufs=2) as p_fcb, \
             tc.tile_pool(name="p_ctx", bufs=2) as p_ctx, \
             tc.tile_pool(name="p_hb", bufs=2) as p_hb, \
             tc.tile_pool(name="p_att", bufs=2) as p_att, \
             tc.tile_pool(name="p_yt", bufs=2) as p_yt, \
             tc.tile_pool(name="ps_o", bufs=2, space="PSUM") as ps_o, \
             tc.tile_pool(name="ps_m", bufs=2, space="PSUM") as ps_m, \
             tc.tile_pool(name="ps_s", bufs=2, space="PSUM") as ps_s, \
             tc.tile_pool(name="ps_t", bufs=2, space="PSUM") as ps_t:
            hsT = perm.tile([128, KH * TOK], BF, tag="hsT")
            ident = perm.tile([128, 128], BF, tag="ident")
            make_identity(nc, ident[:])
            hsT3 = hsT[:].rearrange("p (k n) -> p k n", k=KH)
            hsT4 = hsT[:].rearrange("p (k t b) -> p k t b", k=KH, b=B)
            hsT_t = hsT[:].rearrange("p (k t b) -> p t k b", k=KH, b=B)

            wxh_sb = perm.tile([128, KH * H], BF, tag="wxh")
            whh_sb = perm.tile([128, KH * H], BF, tag="whh")
            bias_sb = perm.tile([128, KH], F32, tag="bias")
            idx_sb = perm.tile([128, TOK // 16], DT.int16, tag="idx")
            mask_sb = perm.tile([128, 128], BF, tag="mask")
            nc.sync.dma_start(out=wxh_sb[:], in_=wxhT[:])
            nc.sync.dma_start(out=whh_sb[:], in_=whhT[:])
            nc.sync.dma_start(out=bias_sb[:], in_=biasT[:])
            nc.sync.dma_start(out=idx_sb[:], in_=idxw[:])
            nc.gpsimd.dma_start(out=mask_sb[:], in_=maskT[:])

            # per-block u tiles (bufs=2): ub[q] holds xproj for block q
            u_tiles = {}
            xe_tiles = {}

            def u_tile(q):
                if q not in u_tiles:
                    t = p_u.tile([128, KH * TB], BF, tag="ub")
                    u_tiles[q] = (t, t[:].rearrange("p (k n) -> p k n", k=KH))
                return u_tiles[q]

            def emit_gather(c):
                t = p_xe.tile([128, KH * NT], BF, tag="xec")
                xe_tiles[c] = (t, t[:].rearrange("p (k n) -> p k n", k=KH))
                nc.gpsimd.dma_gather(
                    out_ap=xe_tiles[c][0][:].rearrange(
                        "p (k n) -> p k n", k=KH),
                    in_ap=emb_bf[:],
                    idxs_ap=idx_sb[:, c * (NT // 16):(c + 1) * (NT // 16)],
                    num_idxs=NT,
                    num_idxs_reg=NT,
                    elem_size=H,
                    transpose=True,
                    single_packet=False,
                )

            def emit_xproj(c):
                q = c // CPB
                cc = c % CPB
                _, ub3 = u_tile(q)
                _, xe3 = xe_tiles[c]
                for mg in range(KH):
                    px = ps_o.tile([128, VW], F32, tag="po", name="px")
                    for k in range(KH):
                        nc.tensor.matmul(
                            px[:],
                            lhsT=wxh_sb[:, k * H + mg * 128:k * H + mg * 128 + 128],
                            rhs=xe3[:, k, :],
                            start=(k == 0), stop=(k == KH - 1),
                        )
                    nc.scalar.activation(
                        ub3[:, mg, cc * NT:(cc + 1) * NT], px[:],
                        mybir.ActivationFunctionType.Identity,
                        bias=bias_sb[:, mg:mg + 1],
                    )
                del xe_tiles[c]

            def emit_rnn_step(t):
                q = t // BLK
                tl = t % BLK
                _, ub3 = u_tile(q)
                if t == 0:
                    nc.scalar.activation(
                        hsT_t[:, 0], ub3[:, :, 0:B],
                        mybir.ActivationFunctionType.Tanh)
                    return
                prev = slice((t - 1) * B, t * B)
                pm = ps_m.tile([128, KH * B], F32, tag="pm")
                pm2 = pm[:].rearrange("p (k b) -> p k b", k=KH)
                # Block 0 has no fc work to hide engine-hop latency behind, so
                # inject u via an identity matmul (PE-only critical chain).
                # Later blocks use a DVE add instead, saving PE time where PE
                # is the bottleneck.
                use_ident = (q == 0)
                for mg in range(KH):
                    if use_ident:
                        nc.tensor.matmul(
                            pm2[:, mg], lhsT=ident[:],
                            rhs=ub3[:, mg, tl * B:(tl + 1) * B],
                            start=True, stop=False)
                    for k in range(KH):
                        nc.tensor.matmul(
                            pm2[:, mg],
                            lhsT=whh_sb[:, k * H + mg * 128:k * H + mg * 128 + 128],
                            rhs=hsT3[:, k, prev],
                            start=(not use_ident and k == 0),
                            stop=(k == KH - 1))
                if use_ident:
                    nc.scalar.activation(
                        hsT_t[:, t], pm2[:],
                        mybir.ActivationFunctionType.Tanh)
                else:
                    hz = p_hz.tile([128, KH * B], BF, tag="hz")
                    nc.vector.tensor_tensor(
                        out=hz[:], in0=pm[:],
                        in1=ub3[:, :, tl * B:(tl + 1) * B],
                        op=mybir.AluOpType.add)
                    nc.scalar.activation(
                        hsT_t[:, t], hz[:].rearrange("p (k b) -> p k b", k=KH),
                        mybir.ActivationFunctionType.Tanh)

            ctx_tiles = {}

            def copy_eng(b):
                # alternate psum->sbuf copies between DVE and ACT so the
                # attention clump doesn't saturate DVE
                return nc.vector if b % 2 == 0 else nc.scalar

            def eng_copy(eng, out, in_):
                if eng is nc.vector:
                    eng.tensor_copy(out, in_)
                else:
                    eng.activation(out, in_,
                                   mybir.ActivationFunctionType.Identity)

            def emit_attn_b(mq, b):
                ntk = (mq + 1) * 128
                if mq not in ctx_tiles:
                    t = p_ctx.tile([128, KH * TB], BF, tag="ctxb")
                    ctx_tiles[mq] = (
                        t,
                        t[:].rearrange("p (k n) -> p k n", k=KH),
                        t[:].rearrange("p (k t b) -> p k t b", k=KH, b=B),
                    )
                _, _, ctxb4 = ctx_tiles[mq]
                # transpose hs blocks 0..mq for batch b into per-b staging
                # (recomputed per block: cheaper in SBUF than retaining the
                # full transposed hs, which would cost another 64KB/part)
                hb = p_hb.tile([128, KH * 512], BF, tag="hb")
                hb3 = hb[:].rearrange("p (c h) -> p c h", c=KH)
                # past blocks: read retained transposed hs back from DRAM
                for ktk in range(mq):
                    eng = (nc.sync, nc.scalar, nc.gpsimd)[(b + ktk) % 3]
                    eng.dma_start(out=hb3[:, ktk], in_=hsat_r[:, ktk, b, :])
                # current block: transpose on PE, stage, and retain to DRAM
                ptt = ps_t.tile([128, KH * 128], BF, tag="pmisc", name="ptt")
                ptt3 = ptt[:].rearrange("p (k n) -> p k n", k=KH)
                for kh in range(KH):
                    nc.tensor.transpose(
                        ptt3[:, kh],
                        hsT4[:, kh, mq * 128:(mq + 1) * 128, b],
                        ident[:])
                eng_copy(copy_eng(b), hb3[:, mq], ptt[:])
                if mq < NBLK - 1:
                    eng = (nc.sync, nc.scalar, nc.gpsimd)[b % 3]
                    eng.dma_start(out=hsat_r[:, mq, b, :], in_=hb3[:, mq])
                # scores (causal skip)
                ps = ps_s.tile([128, S], F32, tag="ps")
                for kh in range(KH):
                    nc.tensor.matmul(
                        ps[:, 0:ntk],
                        lhsT=hsT4[:, kh, mq * 128:(mq + 1) * 128, b],
                        rhs=hsT4[:, kh, 0:ntk, b],
                        start=(kh == 0), stop=(kh == KH - 1))
                nc.vector.tensor_tensor(
                    out=ps[:, mq * 128:ntk], in0=ps[:, mq * 128:ntk],
                    in1=mask_sb[:], op=mybir.AluOpType.add)
                st = p_att.tile([128, 4], F32, tag="st")
                nmx, zs, zi = st[:, 0:1], st[:, 1:2], st[:, 2:3]
                nc.vector.reduce_max(
                    nmx, ps[:, 0:ntk], axis=mybir.AxisListType.X, negate=True)
                w_sb = p_att.tile([128, S], BF, tag="w_sb")
                nc.scalar.activation(
                    w_sb[:, 0:ntk], ps[:, 0:ntk],
                    mybir.ActivationFunctionType.Exp,
                    bias=nmx, accum_out=zs)
                nc.vector.reciprocal(zi, zs)
                nc.vector.tensor_scalar_mul(w_sb[:, 0:ntk], w_sb[:, 0:ntk], zi)
                # transpose w chunks -> wT [tk-part, 128 tq]
                wT = p_att.tile([128, KH * 128], BF, tag="wT")
                wT3 = wT[:].rearrange("p (c n) -> p c n", c=KH)
                for ktk in range(mq + 1):
                    pt = ps_t.tile([128, KH * 128], BF, tag="pmisc", name="pt")
                    nc.tensor.transpose(
                        pt[:, 0:128], w_sb[:, ktk * 128:(ktk + 1) * 128],
                        ident[:])
                    eng_copy(copy_eng(b), wT3[:, ktk, :], pt[:, 0:128])
                # contextT block cols for b
                pc = ps_t.tile([128, KH * 128], F32, tag="pmisc", name="pc")
                pc3 = pc[:].rearrange("p (k n) -> p k n", k=KH)
                for mh in range(KH):
                    for ktk in range(mq + 1):
                        nc.tensor.matmul(
                            pc3[:, mh],
                            lhsT=hb3[:, ktk, mh * 128:(mh + 1) * 128],
                            rhs=wT3[:, ktk, :],
                            start=(ktk == 0), stop=(ktk == mq))
                eng_copy(copy_eng(b), ctxb4[:, :, :, b], pc3[:, :, :])

            fw_tiles = {}
            fcb_tiles = {}

            def emit_fw_load(q, vb):
                fw = p_fw.tile([128, KD * VW], BF, tag="fw")
                fw_tiles[(q, vb)] = (fw, fw[:].rearrange("p (k v) -> p k v", k=KD))
                nc.sync.dma_start(out=fw[:], in_=fcwT3[:, vb, :])
                fcbc = p_fcb.tile([128, VW], BF, tag="fcbc")
                fcb_tiles[(q, vb)] = fcbc
                nc.sync.dma_start(
                    out=fcbc[:], in_=fcb_in[:, vb * VW:(vb + 1) * VW])

            yt_tiles = {}

            def emit_fc_group(q, vb, mtl):
                _, _, _ = 0, 0, 0
                _, fw3 = fw_tiles[(q, vb)]
                fcbc = fcb_tiles[(q, vb)]
                _, ctxb3, _ = ctx_tiles[q]
                mt = q * (TB // 128) + mtl
                po = ps_o.tile([128, VW], F32, tag="po", name="po")
                for k in range(KD):
                    lhsT = (hsT3[:, k, mt * 128:(mt + 1) * 128]
                            if k < KH else
                            ctxb3[:, k - KH, mtl * 128:(mtl + 1) * 128])
                    nc.tensor.matmul(
                        po[:], lhsT=lhsT, rhs=fw3[:, k],
                        start=(k == 0), stop=(k == KD - 1))
                og = mtl // OB
                if (q, vb, og) not in yt_tiles:
                    yt_tiles[(q, vb, og)] = p_yt.tile(
                        [128, OB * VW], BF, tag="yt", name="yt")
                yt = yt_tiles[(q, vb, og)]
                ol = mtl % OB
                nc.vector.tensor_tensor(
                    out=yt[:, ol * VW:(ol + 1) * VW], in0=po[:],
                    in1=fcbc[:], op=mybir.AluOpType.add)

            store_eng = [0]

            def emit_store(q, vb, og):
                yt = yt_tiles.pop((q, vb, og))
                mt0 = q * (TB // 128) + og * OB
                eng = nc.sync if store_eng[0] % 2 == 0 else nc.gpsimd
                store_eng[0] += 1
                eng.dma_start(
                    out=y_r[:, vb, mt0:mt0 + OB, :],
                    in_=yt[:].rearrange("p (m v) -> p m v", m=OB))

            def attn_items(qm):
                return [("attn", lambda mq=qm, b=b: emit_attn_b(mq, b))
                        for b in range(B)]

            def fc_items(qm, vbs, early_fw=2):
                items = []
                for vb in vbs[:early_fw]:
                    items.append(("fw", lambda q=qm, vb=vb: emit_fw_load(q, vb)))
                for i, vb in enumerate(vbs):
                    if i >= early_fw:
                        items.append(
                            ("fw", lambda q=qm, vb=vb: emit_fw_load(q, vb)))
                    for og in range(TB // 128 // OB):
                        for ol in range(OB):
                            items.append(
                                ("fc", lambda q=qm, vb=vb, mtl=og * OB + ol:
                                 emit_fc_group(q, vb, mtl)))
                        items.append(
                            ("st", lambda q=qm, vb=vb, og=og:
                             emit_store(q, vb, og)))
                return items

            def block_work(qm, vbs):
                """Work items for block qm: early fw loads, attention
                (paced 1/step), then fc groups + stores."""
                fitems = fc_items(qm, vbs)
                return fitems[:2] + attn_items(qm) + fitems[2:]

            # ---------------- emission schedule ----------------
            xg = [0, 0]   # next gather, next xproj

            def pump_xproj(n, qmax):
                for _ in range(n):
                    if xg[0] < NCH and xg[0] <= xg[1] + 1 \
                            and xg[0] // CPB <= qmax + 1:
                        emit_gather(xg[0])
                        xg[0] += 1
                    if xg[1] < NCH and xg[1] < xg[0] \
                            and xg[1] // CPB <= qmax:
                        emit_xproj(xg[1])
                        xg[1] += 1

            pump_xproj(2, 0)
            work = []
            for q in range(NBLK):
                if q == 0:
                    work = []
                elif q < NBLK - 1:
                    work = block_work(q - 1, list(range(NVB)))
                else:
                    # hold back the last 2 vb chunks of block 2's fc to
                    # interleave with block 3's attention in the tail
                    work = block_work(q - 1, list(range(NVB - 2)))
                wi = 0
                for cc in range(CPB):
                    pump_xproj(2, q + 1)
                    for tl in range(BLK // CPB):
                        t = (q * CPB + cc) * (BLK // CPB) + tl
                        emit_rnn_step(t)
                        steps_left = BLK - (cc * (BLK // CPB) + tl)
                        want = min(
                            -(-(len(work) - wi) // max(steps_left, 1)), 4)
                        popped = 0
                        while wi < len(work) and popped < want:
                            kind, fn = work[wi]
                            fn()
                            wi += 1
                            popped += 1
                            if kind == "attn":
                                break
                while wi < len(work):
                    work[wi][1]()
                    wi += 1
            # tail: interleave block 3 attention with block 2's held-back fc
            leftover = fc_items(NBLK - 2, [NVB - 2, NVB - 1])
            a3 = attn_items(NBLK - 1)
            li = 0
            for i, (_, it) in enumerate(a3):
                it()
                take = -(-(len(leftover) - li) // (len(a3) - i))
                for _ in range(min(take, 4)):
                    if li < len(leftover):
                        leftover[li][1]()
                        li += 1
            while li < len(leftover):
                leftover[li][1]()
                li += 1
            for _, it in fc_items(NBLK - 1, list(range(NVB))):
                it()

    nc.compile()
    return nc


# ---------------------------------------------------------------------------
# host side
# ---------------------------------------------------------------------------

def prep_inputs(x, emb, Wxh_w, Wxh_b, Whh_w, Whh_b, fc_w, fc_b):
    """Build per-core in_maps with device layouts."""
    x = np.asarray(x)
    emb = np.asarray(emb, dtype=np.float32)
    Wxh_w = np.asarray(Wxh_w, dtype=np.float32)
    Wxh_b = np.asarray(Wxh_b, dtype=np.float32)
    Whh_w = np.asarray(Whh_w, dtype=np.float32)
    Whh_b = np.asarray(Whh_b, dtype=np.float32)
    fc_w = np.asarray(fc_w, dtype=np.float32)
    fc_b = np.asarray(fc_b, dtype=np.float32)

    emb_bf = np.ascontiguousarray(emb.astype(BF_NP))
    # idx wrapped: flat tok order = t*16+b ; slot j -> [j%16, j//16]
    idx_flat = np.ascontiguousarray(x.T).reshape(-1).astype(np.int64)
    wrapped = idx_flat.reshape(TOK // 16, 16).T.astype(np.int16)
    idxw = np.ascontiguousarray(np.tile(wrapped, (8, 1)))

    def pack_T(w):  # w [G, H] -> lhsT layout [128, KH*G]
        wT = np.ascontiguousarray(w.T)            # [H, G]
        kh = wT.shape[0] // 128
        return np.ascontiguousarray(
            wT.reshape(kh, 128, wT.shape[1]).transpose(1, 0, 2).reshape(128, -1)
        ).astype(BF_NP)

    wxhT = pack_T(Wxh_w)
    whhT = pack_T(Whh_w)
    bias = (Wxh_b + Whh_b).astype(np.float32)
    biasT = np.ascontiguousarray(bias.reshape(KH, 128).T)

    p = np.arange(128)[:, None]
    j = np.arange(128)[None, :]
    maskT = np.where(j <= p, 0.0, -1e30).astype(np.float32)

    base = {
        "emb_bf": emb_bf, "idxw": idxw, "wxhT": wxhT, "whhT": whhT,
        "biasT": biasT, "maskT": maskT,
    }
    in_maps = []
    for c in range(NCORES):
        sl = slice(c * VS, (c + 1) * VS)
        fcwT_kv = pack_T(fc_w[sl]).reshape(128, KD, VS)   # [p, k, v]
        fcwT = np.zeros((128, NVB, KD, VW), BF_NP)
        for vb in range(NVB):
            vw = min(VW, VS - vb * VW)
            fcwT[:, vb, :, :vw] = fcwT_kv[:, :, vb * VW:vb * VW + vw]
        fcwT = np.ascontiguousarray(fcwT.reshape(128, NVB * KD * VW))
        fcb_pad = np.zeros(NVB * VW, np.float32)
        fcb_pad[:VS] = fc_b[sl]
        fcb_bc = np.ascontiguousarray(
            np.broadcast_to(fcb_pad.astype(BF_NP), (128, NVB * VW)))
        m = dict(base)
        m["fcwT"] = fcwT
        m["fcb"] = fcb_bc
        in_maps.append(m)
    return in_maps


def unpack_y(res_y_list):
    """res_y_list: per-core y [NVB, MTB, 128, VW] bf16 -> [B, S, V] f32."""
    out = np.empty((B, S, VOCAB), np.float32)
    for c, yd in enumerate(res_y_list):
        yc = np.asarray(yd)                       # [NVB, MTB, 128, VW]
        # tok = mt*128 + p ; tok = t*16 + b
        for vb in range(NVB):
            vw = min(VW, VS - vb * VW)
            v0 = c * VS + vb * VW
            blk = yc[vb].reshape(TOK, VW)[:, :vw].astype(np.float32)
            out[:, :, v0:v0 + vw] = blk.reshape(S, B, vw).transpose(1, 0, 2)
    return out


_NC_CACHE = {}


def get_nc(repeat=1):
    if repeat not in _NC_CACHE:
        _NC_CACHE[repeat] = build_nc(repeat)
    return _NC_CACHE[repeat]


def kernel(x, emb, Wxh_w, Wxh_b, Whh_w, Whh_b, fc_w, fc_b):
    nc = get_nc()
    in_maps = prep_inputs(x, emb, Wxh_w, Wxh_b, Whh_w, Whh_b, fc_w, fc_b)
    res = run_bass_kernel_spmd(nc, in_maps, list(range(NCORES)))
    return unpack_y([res.results[c]["y"] for c in range(NCORES)])
